# revision 38
# baseline (speedup 1.0000x reference)
"""Multi-head self-attention (B=4, S=2048, E=1024, H=16) on 8 NeuronCores.

Sharding: batch (4) x head-group (2 groups of 8 heads), one (b, g) pair per
core.  Each core computes Q/K/V projections for its head group, attention,
and a partial output projection (row-parallel over Wo); the host sums the
two head-group partials per batch.

Layout strategy: the host feeds x transposed (xT = x.T, [E, S], bf16) so
every matmul's contraction dim lands on SBUF partitions with no on-chip
transposes.  Scores are computed transposed (scoresT[k, q] = K @ Q^T per
head, two heads packed into disjoint PE row groups), softmax denominators
come free via a ones-column appended to V (attn @ [V|1] with V stationary
directly yields attnT[d, q] plus the denom row - exactly the lhsT the
output projection needs).  exp runs on ScalarE; PSUM->SBUF eviction and
biases on VectorE; denom broadcast on GpSimd.

All operands and the output are bf16 (f32 PSUM accumulation and f32
biases/denoms; the host sums the two head-group partials in f32);
measured end-to-end relative error vs the fp32 reference is ~7e-3
against a 2e-2 budget.  fp8 was evaluated and rejected (7.9e-2).

Schedule: the kernel is PE-bound (NTFF: ~358us tensor-engine active vs
~272us of exp on ACT), so the entire layout serves one goal - the PE
never stalls.  Attention runs in 512-wide q-chunks so PSUM splits into
dedicated pools (scores ping-pong 2x2 banks, AV accumulators 2x1,
projection tiles 2x1) and no pool is shared between the exp pipeline and
filler work.  Projections and the chunk-c output projection ride inside
later chunks' attention kk-loops as filler units sized ~0.9-1.7us; the
lead-in computes only K/Q(m0,s0) - with the wk DMA split so its m=0
slice lands first and x seg0 streaming per k-row - and the first phase
runs a deeper exp->AV skew so the V projection and remaining K tiles
absorb the input-DMA-paced window.  The softmax ones column is memset on
GpSimd (a strided DMA here costs 16K descriptors).

Softmax normalization: the AV PSUM banks are released by a ~0.7us DVE
copy to SBUF right after the last AV matmul; the lane-starved exact
reciprocal (denom row lives on one partition: 3.4us), gpsimd broadcast
and mul then run off the PE critical path.  Holding the banks through
that chain stalled the next phase's first AV matmul ~5us at every phase
boundary (which also re-throttled the PE: HAM un-throttles on activity).

Input DMA: xT/wq/wk/wv/wo arrive host-pre-permuted so each partition's
bytes are one contiguous 8KB DRAM run (wk [128,ET,DG] whole-tensor; xT
[128,NQC,ET,QCH] per-segment).  With the natural [E,*] layouts every
transfer was 1KB (or 256B) runs and the stream ran at ~96GB/s effective
vs ~360GB/s HBM - descriptor-rate-bound at ~150ns/descriptor/SDMA -
which paced the whole first phase.

Measured alternatives that LOST (all on hardware, vs the 431.6us best):
reciprocal_approx_fast (custom-DVE uop table never reaches the NEFF
through this compile path - silently wrong results); av copies on ACT
instead of DVE (+20us: ScalarE SBUF-source ops are ~2.3x slower than
spec, and the boundary copies push the next phase's exp queue into the
AV skew); 1/d as exp(-ln d) on ACT (+60us, same errata: 3.1us/head);
deferring the recip/mul chain into the next phase's filler slots (+21us;
the Tile scheduler reorders outproj units toward the phase start, so
"later" kk placements don't protect them - only making the at tiles
ready earlier does); chunking the norm at 256 (neutral); fp8 e4m3 (max
240) for the exp weights or V, evaluated numerically: each alone costs
~2e-2 max-rel-err against the 2e-2 budget (DoubleRow AV needs both).

NTFF-profiled HW exec (max over 8 cores): 429.8us (rel err 6.29e-3),
down from 494.7us for the previous revision (~95us throttled + ~100us
PE-idle at phase boundaries).  Remaining gap to the ~355us PE-busy
floor: outproj-vs-norm at-tile races (~20us), lead-in/first-phase DMA
pacing (~8us), last-phase tail + final out-DMA drain (~10us).
Wall-clock in this axon environment is ~71ms/call of tunnel round-trip
and measures nothing about the kernel.
"""

import ml_dtypes
import numpy as np

import concourse.bacc as bacc
import concourse.mybir as mybir
import concourse.tile as tile
from concourse.bass_utils import run_bass_kernel_spmd

B, S, E, H = 4, 2048, 1024, 16
GROUPS = 2                 # tensor-parallel head groups
HG = H // GROUPS           # heads per core
DH = E // H                # head dim
DG = HG * DH               # projected dim per core (512)
ET, DT, ST = E // 128, DG // 128, S // 128
QCH = 512                  # q-chunk (one PSUM bank)
NQC = S // QCH
SCALE = 1.0 / np.sqrt(DH)
SKEW = 4                   # exp -> AV pipeline depth (kk steps)
SKEW0 = 6                  # deeper skew for the DMA-paced first phase

f32 = mybir.dt.float32
bf16 = mybir.dt.bfloat16
FT = mybir.ActivationFunctionType

_CACHE = {}


def _body(nc, tc, xT, wq, wk, wv, wo, bqk, bv, bo, out):
    import os as _os
    _mark = (lambda tag: print("MARK", tag, nc.next_id())) if _os.environ.get("KMARK") else (lambda tag: None)
    with tc.tile_pool(name="pers", bufs=1) as pers, \
         tc.tile_pool(name="pp", bufs=1, space="PSUM") as pp:
        qt = pers.tile([128, DT, S], bf16)            # Q^T  [d, s]
        kt = pers.tile([128, DT, S], bf16)            # K^T  [d, s]
        vv = pers.tile([128, ST, HG, DH + 1], bf16)   # V [s, h, d|1]
        xt = pers.tile([128, ET, S], bf16)            # x^T  [e, s]
        wq_sb = pers.tile([128, ET, DG], bf16)
        wk_sb = pers.tile([128, ET, DG], bf16)
        wv_sb = pers.tile([128, ET, DG], bf16)
        wo_sb = pers.tile([128, DT, E], bf16)
        bqk_sb = pers.tile([128, 2 * DT], f32)
        bvbc = pers.tile([128, DG], f32)              # bv broadcast over s
        bobc = pers.tile([128, E], f32)               # bo broadcast over s

        # DMA issue order tracks consumption order: wk + x seg0 (lead-in K),
        # wq (lead-in Q), ones/bv/wv (V units early in the first phase),
        # x seg1-3 (K(m0,s1..3) units + scores kk>=4), then wo/biases.
        # All large tensors arrive pre-permuted from the host so every
        # partition's data is one contiguous 8KB DRAM run: the input stream
        # is descriptor-rate-bound (~150ns/descriptor/SDMA-engine; measured
        # ~96GB/s effective vs ~360GB/s HBM with 1KB runs), and 8x fewer
        # descriptors shortens the DMA-paced first-phase window.
        # wk whole-tensor: the former m=0 column-slice split used 256B
        # descriptors (1024 of them ~ 9.6us at descriptor rate) and became
        # the lead-in head-of-line blocker once everything else was 8KB.
        # wk and x seg0 in ET-halves (4KB/partition runs, still
        # descriptor-efficient): the lead-in K k-loop consumes k=0..7 in
        # order, so its first matmuls start after half the data lands
        # (~3us earlier; the cold-start DMA wait is the remaining gap).
        H2 = ET // 2
        nc.sync.dma_start(out=wk_sb[:, 0:H2, :], in_=wk[:, 0:H2])
        nc.sync.dma_start(out=xt[:, 0:H2, 0:QCH], in_=xT[:, 0, 0:H2])
        nc.sync.dma_start(out=wk_sb[:, H2:ET, :], in_=wk[:, H2:ET])
        nc.sync.dma_start(out=xt[:, H2:ET, 0:QCH], in_=xT[:, 0, H2:ET])
        nc.sync.dma_start(out=bqk_sb, in_=bqk)
        nc.sync.dma_start(out=wq_sb, in_=wq)
        for c in range(1, NQC):
            nc.sync.dma_start(
                out=xt[:, :, c * QCH:(c + 1) * QCH],
                in_=xT[:, c],
            )
        with nc.allow_low_precision(reason="ones column for denom"):
            nc.gpsimd.memset(vv[:, :, :, DH:DH + 1], 1.0)
        nc.sync.dma_start(out=bvbc, in_=bv.to_broadcast((128, DG)))
        nc.sync.dma_start(out=wv_sb, in_=wv)
        nc.sync.dma_start(out=wo_sb, in_=wo)
        nc.sync.dma_start(out=bobc, in_=bo.to_broadcast((128, E)))

        def qk_proj(wsb, dst, ip, m, c):
            ps = pp.tile([128, QCH], f32, tag="mm", bufs=2, name="ps_qk")
            for k in range(ET):
                nc.tensor.matmul(
                    ps,
                    wsb[:, k, m * 128:(m + 1) * 128],
                    xt[:, k, c * QCH:(c + 1) * QCH],
                    start=(k == 0),
                    stop=(k == ET - 1),
                )
            with nc.allow_low_precision(reason="bf16 activations for PE"):
                nc.vector.tensor_scalar_add(
                    dst[:, m, c * QCH:(c + 1) * QCH], ps,
                    bqk_sb[:, ip * DT + m:ip * DT + m + 1],
                )

        def v_proj(ms):
            ps = pp.tile([128, QCH], f32, tag="mm", bufs=2, name="ps_v")
            for k in range(ET):
                nc.tensor.matmul(
                    ps,
                    xt[:, k, ms * 128:(ms + 1) * 128],
                    wv_sb[:, k, :],
                    start=(k == 0),
                    stop=(k == ET - 1),
                )
            with nc.allow_low_precision(reason="bf16 V for PE"):
                nc.vector.tensor_add(
                    vv[:, ms, :, 0:DH],
                    ps.rearrange("p (h d) -> p h d", h=HG),
                    bvbc.rearrange("p (h d) -> p h d", h=HG),
                )

        with tc.tile_pool(name="p3", bufs=1) as p3:
            ats = {}

            def get_at(c):
                if c not in ats:
                    # 3 chunks alive: chunk-c halves are read into chunk c+2
                    ats[c] = [p3.tile([128, QCH], bf16, tag="attnT",
                                      bufs=3 * DT, name=f"at{c}_{j}")
                              for j in range(DT)]
                return ats[c]

            def attention_kk(c, pr, extra_units, skew=SKEW):
                avs = [pp.tile([128, QCH], f32, tag="av", bufs=2,
                               name=f"av{i}") for i in range(2)]
                exq = {}
                for kk in range(ST + skew):
                    if kk < ST:
                        # one 2-bank tile holds both heads' scores so exp can
                        # run 1024 wide (halves ACT per-inst overhead)
                        sc = pp.tile([128, 2 * QCH], f32, tag="sc", bufs=2,
                                     name="sc")
                        for i in range(2):
                            o = i * 64
                            nc.tensor.matmul(
                                sc[:, i * QCH:(i + 1) * QCH],
                                kt[o:o + 64, pr, kk * 128:(kk + 1) * 128],
                                qt[o:o + 64, pr, c * QCH:(c + 1) * QCH],
                                start=True,
                                stop=True,
                            )
                        ex = p3.tile([128, 2 * QCH], bf16, tag="expt",
                                     bufs=2 * (SKEW0 + 1), name="ex")
                        nc.scalar.activation(
                            out=ex, in_=sc, func=FT.Exp, scale=SCALE
                        )
                        exq[kk] = ex
                    for u in extra_units.get(kk, ()):
                        u()
                    k2 = kk - skew
                    if k2 >= 0:
                        ex = exq.pop(k2)
                        for i, av in enumerate(avs):
                            h = 2 * pr + i
                            nc.tensor.matmul(
                                av[0:DH + 1, :],
                                vv[:, k2, h],
                                ex[:, i * QCH:(i + 1) * QCH],
                                start=(k2 == 0),
                                stop=(k2 == ST - 1),
                            )
                return avs

            def norm_copy(avs):
                # Release the av PSUM banks ~0.7us after the last AV matmul
                # via ACT copies to SBUF (ACT idles at the boundary - its
                # exp queue just drained).  Holding av through the full
                # recip/broadcast/mul chain (the lane-starved exact recip
                # alone is 3.4us: the denom row lives on one partition)
                # stalled the next phase's first AV matmul ~5us at every
                # boundary (NTFF-measured).
                avf = [p3.tile([DH + 1, QCH], f32, tag="avf", bufs=4,
                               name="avf") for _ in avs]
                for i, av in enumerate(avs):
                    nc.vector.tensor_copy(avf[i], av[0:DH + 1, :])
                return avf

            def norm_finish(avf, c, pr, chunk=QCH):
                # Chunked: the recip/bcast/mul chain runs per column-chunk
                # so the first at columns complete early (the exact recip
                # is lane-starved, so latency scales with chunk width).
                # NOTE: computing 1/d as exp(-ln(d)) on ACT was measured
                # WORSE (+60us total): ScalarE SBUF-source ops run ~2.3x
                # slower than spec (cayman errata), so the pair costs
                # 3.1us/head and pushes the next phase's exp queue into
                # the AV skew.  A fused [2,512] recip over both heads'
                # denom rows (copied adjacent) fails to compile through
                # this path - left on the table.
                at = get_at(c)
                bcs = [p3.tile([64, QCH], f32, tag="bc", bufs=4, name="bc")
                       for _ in avf]
                for q0 in range(0, QCH, chunk):
                    sl = slice(q0, q0 + chunk)
                    for i in range(len(avf)):
                        nc.vector.reciprocal(out=bcs[i][0:1, sl],
                                             in_=avf[i][DH:DH + 1, sl])
                    for i in range(len(avf)):
                        nc.gpsimd.partition_broadcast(out_ap=bcs[i][:, sl],
                                                      in_ap=bcs[i][0:1, sl])
                    for i in range(len(avf)):
                        h = 2 * pr + i
                        o = (h % 2) * 64
                        with nc.allow_low_precision(reason="bf16 attn for PE"):
                            nc.vector.tensor_mul(
                                at[h // 2][o:o + 64, sl], avf[i][0:DH, sl],
                                bcs[i][:, sl]
                            )

            def outproj_half(c, ms, nn):
                at = get_at(c)
                lo, hi = nn * 512, (nn + 1) * 512
                po = pp.tile([128, 512], f32, tag="mm", bufs=2, name="po")
                for j in range(DT):
                    nc.tensor.matmul(
                        po,
                        at[j][:, ms * 128:(ms + 1) * 128],
                        wo_sb[:, j, lo:hi],
                        start=(j == 0),
                        stop=(j == DT - 1),
                    )
                ou = p3.tile([128, 512], bf16, tag="out", bufs=4, name="ou")
                with nc.allow_low_precision(reason="bf16 output"):
                    nc.vector.tensor_add(ou, po, bobc[:, lo:hi])
                r0 = c * QCH + ms * 128
                nc.sync.dma_start(out=out[r0:r0 + 128, lo:hi], in_=ou)

            # Filler units per (c, pr) phase: kk index -> [callables].
            def ext(c, pr):
                u = {}

                def add(kk, fn):
                    u.setdefault(kk, []).append(fn)

                if c == 0:
                    if pr == 0:
                        # DMA-paced phase, AV skew 6: K(m0,s1..3) first
                        # (scores kk>=4 need them), V pairs on odd kks
                        # (vv[j] due at kk=j+SKEW0), K(m1)/Q(m1) on even
                        # kks for phase (0,1).
                        for s in range(1, NQC):
                            add(s, lambda s=s: qk_proj(wk_sb, kt, 1, 0, s))
                        for j in range(ST):
                            add(2 * (j // 2) + 5, lambda j=j: v_proj(j))
                        # K(m1,s0) moved into the lead-in: its inputs (wk,
                        # x seg0) are resident while the PE otherwise idles
                        # ~7us waiting for the wq DMA before Q(m0,c0).
                        for s in range(1, NQC):
                            add(6 + 2 * s, lambda s=s: qk_proj(wk_sb, kt, 1, 1, s))
                        add(14, lambda: qk_proj(wq_sb, qt, 0, 1, 0))
                    elif pr < DT - 1:
                        m = pr + 1
                        for s in range(NQC):
                            add(2 + 4 * s, lambda m=m, s=s: qk_proj(wk_sb, kt, 1, m, s))
                        add(16, lambda m=m: qk_proj(wq_sb, qt, 0, m, 0))
                    else:
                        for m in range(DT):
                            add(2 + 4 * m, lambda m=m: qk_proj(wq_sb, qt, 0, m, 1))
                elif c == 1:
                    # pr==0: outproj(c-1,0,*) reads at[c-1][3], whose norm
                    # muls land ~+10us (~kk 8) after the boundary; kk=3
                    # raced them (6.3us LDWEIGHTS stall, NTFF-measured).
                    add(10 if pr == 0 else 3,
                        lambda pr=pr: outproj_half(0, pr, 0))
                    add(9, lambda pr=pr: qk_proj(wq_sb, qt, 0, pr, 2))
                elif c == 2:
                    add(3, lambda pr=pr: outproj_half(0, pr, 1))
                    add(9, lambda pr=pr: qk_proj(wq_sb, qt, 0, pr, 3))
                    add(13, lambda pr=pr: outproj_half(1, pr, 0))
                else:
                    add(3, lambda pr=pr: outproj_half(1, pr, 1))
                    add(10 if pr == 0 else 8,
                        lambda pr=pr: outproj_half(2, pr, 0))
                    add(13, lambda pr=pr: outproj_half(2, pr, 1))
                return u

            # Lead-in: what att(0,0) strictly needs from the PE, plus
            # K(m1,s0) as backfill for the wq-DMA wait (its inputs arrive
            # with K(m0,s0)'s).
            _mark('leadin')
            qk_proj(wk_sb, kt, 1, 0, 0)
            qk_proj(wk_sb, kt, 1, 1, 0)
            qk_proj(wq_sb, qt, 0, 0, 0)

            for c in range(NQC):
                for pr in range(HG // 2):
                    _mark(f'att_kk({c},{pr})')
                    if (c, pr) == (0, 0):
                        skew = SKEW0        # ride out the input-DMA window
                    elif (c, pr) == (NQC - 1, HG // 2 - 1):
                        skew = 2            # last phase: earlier norm/tail
                    else:
                        skew = SKEW
                    avs = attention_kk(c, pr, ext(c, pr), skew=skew)
                    _mark(f'norm({c},{pr})')
                    avf = norm_copy(avs)
                    # chunk=256 for non-last phases was measured neutral
                    # (+1us, more DVE/gpsimd instructions) - keep 512.
                    norm_finish(avf, c, pr, chunk=128 if skew == 2 else QCH)
            _mark('outproj_tail')
            for ms in range(QCH // 128):
                for nn in range(E // 512):
                    outproj_half(NQC - 1, ms, nn)


def _declare(nc):
    # xT/wq/wv/wo are host-pre-permuted so each partition's bytes are one
    # contiguous DRAM run (see _in_maps); wk stays [E, DG] for the lead-in
    # column-slice split.
    xT = nc.dram_tensor("xT", [128, NQC, ET, QCH], bf16,
                        kind="ExternalInput").ap()
    wq = nc.dram_tensor("wq", [128, ET, DG], bf16, kind="ExternalInput").ap()
    wk = nc.dram_tensor("wk", [128, ET, DG], bf16, kind="ExternalInput").ap()
    wv = nc.dram_tensor("wv", [128, ET, DG], bf16, kind="ExternalInput").ap()
    wo = nc.dram_tensor("wo", [128, DT, E], bf16, kind="ExternalInput").ap()
    bqk = nc.dram_tensor("bqk", [128, 2 * DT], f32, kind="ExternalInput").ap()
    bv = nc.dram_tensor("bv", [1, DG], f32, kind="ExternalInput").ap()
    bo = nc.dram_tensor("bo", [1, E], f32, kind="ExternalInput").ap()
    out = nc.dram_tensor("out", [S, E], bf16, kind="ExternalOutput").ap()
    return xT, wq, wk, wv, wo, bqk, bv, bo, out


def _build():
    nc = bacc.Bacc("TRN2", target_bir_lowering=False, debug=False)
    args = _declare(nc)
    with tile.TileContext(nc) as tc:
        _body(nc, tc, *args)
    nc.compile()
    return nc


def _pack_w(w):
    # [E, N] -> [128, E//128, N]: one contiguous 8KB DRAM run per partition.
    return np.ascontiguousarray(w.reshape(E // 128, 128, -1).transpose(1, 0, 2))


def _in_maps(inputs):
    x = np.asarray(inputs["inputs"], np.float32)
    b16 = ml_dtypes.bfloat16
    maps = []
    for b in range(B):
        xT = np.ascontiguousarray(x[b].T.astype(b16))
        # [E, S] -> [128, NQC, ET, QCH]: xTp[p, c, a, s] = xT[a*128+p,
        # c*QCH+s], so each (partition, segment) is one 8KB run.
        xTp = np.ascontiguousarray(
            xT.reshape(ET, 128, NQC, QCH).transpose(1, 2, 0, 3))
        for g in range(GROUPS):
            sl = slice(g * DG, (g + 1) * DG)
            bq_g = np.asarray(inputs["bq"], np.float32)[sl]
            bk_g = np.asarray(inputs["bk"], np.float32)[sl]
            bqk = np.concatenate(
                [bq_g.reshape(DT, 128).T, bk_g.reshape(DT, 128).T], axis=1
            )
            wo_g = np.asarray(inputs["Wo"], np.float32)[sl, :].astype(b16)
            maps.append({
                "xT": xTp,
                "wq": _pack_w(np.asarray(inputs["Wq"], np.float32)[:, sl].astype(b16)),
                "wk": _pack_w(np.asarray(inputs["Wk"], np.float32)[:, sl].astype(b16)),
                "wv": _pack_w(np.asarray(inputs["Wv"], np.float32)[:, sl].astype(b16)),
                "wo": np.ascontiguousarray(
                    wo_g.reshape(DT, 128, E).transpose(1, 0, 2)),
                "bqk": np.ascontiguousarray(bqk),
                "bv": np.asarray(inputs["bv"], np.float32)[sl].reshape(1, DG),
                "bo": np.asarray(inputs["bo"], np.float32).reshape(1, E),
            })
    return maps


def kernel(**inputs) -> np.ndarray:
    if "nc" not in _CACHE:
        _CACHE["nc"] = _build()
    nc = _CACHE["nc"]
    res = run_bass_kernel_spmd(nc, _in_maps(inputs), core_ids=list(range(B * GROUPS)))
    out = np.zeros((B, S, E), np.float32)
    for b in range(B):
        out[b] = (res.results[2 * b]["out"].astype(np.float32)
                  + res.results[2 * b + 1]["out"].astype(np.float32))
    return out

